# revision 1
# baseline (speedup 1.0000x reference)
"""Causal self-attention Trainium2 kernel.

Full computation: y = softmax_causal((x@Wq)(x@Wk)^T / sqrt(D)) @ (x@Wv) @ Wp
Sharding: head-parallel over 8 cores (H=8 heads, one per core), both batches
on every core (batch 0 on SBUF partitions 0:64, batch 1 on 64:128).
Each core produces a partial output (its head's contribution to y @ W_proj);
the host sums the 8 partials.
"""

import sys

sys.path.insert(0, "/opt/trn_rl_repo")

from contextlib import ExitStack

import numpy as np

import concourse.bass as bass
import concourse.mybir as mybir
import concourse.tile as tile
from concourse import bacc

B, T, C, H, D = 2, 4096, 512, 8, 64
BT = B * T  # 8192
NCORES = 8
NC_CH = C // 128  # 4 contraction chunks for the QKV projection
NQT = T // 512  # 8 q-tiles per batch
NKT = T // 128  # 32 k-tiles per batch
KGRP = 3  # k-tiles per exp group (3 PSUM banks, double buffered)

f32 = mybir.dt.float32
f32r = mybir.dt.float32r
bf16 = mybir.dt.bfloat16


def _r(ap):
    return ap  # tiles are fp32r-typed now


def build_kernel() -> bass.Bass:
    nc = bacc.Bacc()

    xT = nc.dram_tensor("xT", [C, BT], bf16, kind="ExternalInput")
    wq = nc.dram_tensor("wq", [C, D], bf16, kind="ExternalInput")
    wk = nc.dram_tensor("wk", [C, D], bf16, kind="ExternalInput")
    wv = nc.dram_tensor("wv", [C, D], bf16, kind="ExternalInput")
    # wp row D is zeros; rows 0:D are this head's W_proj slice.
    wp = nc.dram_tensor("wp", [D + 1, C], f32r, kind="ExternalInput")
    ev = nc.dram_tensor("ev", [D + 1, 2], f32r, kind="ExternalInput")
    ones64 = nc.dram_tensor("ones64", [64], f32r, kind="ExternalInput")
    outp = nc.dram_tensor("outp", [BT, C], f32, kind="ExternalOutput")

    xTr = xT[:, :].rearrange("(a p) t -> a p t", p=128)  # [4, 128, BT]

    with tile.TileContext(nc) as tc, ExitStack() as ctx:
        singles = ctx.enter_context(tc.tile_pool(name="singles", bufs=1))

        # Persistent SBUF tensors
        qT = singles.tile([128, T], f32r)  # [0:64]=batch0 head dims, [64:128]=batch1
        kT = singles.tile([128, T], f32r)
        v_sb = singles.tile([128, B * NKT, D + 1], f32r)  # v tiles + ones column
        yT = singles.tile([D + 1, BT], f32r)  # unnormalized y^T; row D = softmax sums
        wq_sb = singles.tile([128, NC_CH, D], bf16)
        wk_sb = singles.tile([128, NC_CH, D], bf16)
        wv_sb = singles.tile([128, NC_CH, D], bf16)
        wp_sb = singles.tile([D + 1, C], f32r)
        e_sb = singles.tile([D + 1, 2], f32r)

        nc.sync.dma_start(wq_sb[:], wq[:, :].rearrange("(a p) d -> p a d", p=128))
        nc.sync.dma_start(wk_sb[:], wk[:, :].rearrange("(a p) d -> p a d", p=128))
        nc.sync.dma_start(wv_sb[:], wv[:, :].rearrange("(a p) d -> p a d", p=128))
        nc.sync.dma_start(wp_sb[:], wp[:, :])
        nc.sync.dma_start(e_sb[:], ev[:, :])
        o = ones64[:]
        ones_bcast = bass.AP(tensor=o.tensor, offset=o.offset, ap=[[0, 128], [1, 64]])
        nc.gpsimd.dma_start(out=v_sb[:, :, D], in_=ones_bcast)

        # ---------------- Phase 1: QKV projection ----------------
        with (
            tc.tile_pool(name="p1x", bufs=4) as xpool,
            tc.tile_pool(name="p1qk", bufs=2, space="PSUM") as psqk,
            tc.tile_pool(name="p1v", bufs=4, space="PSUM") as psv,
        ):
            for j in range(NQT):
                for b in range(B):
                    t0 = b * T + j * 512
                    xt = xpool.tile([128, NC_CH, 512], bf16, tag="xt")
                    for c in range(NC_CH):
                        nc.sync.dma_start(xt[:, c, :], xTr[c, :, t0 : t0 + 512])
                    lo, hi = 64 * b, 64 * b + 64
                    tp = (0, 64) if b == 1 else None
                    pq = psqk.tile([128, 512], f32, tag="pq")
                    pk = psqk.tile([128, 512], f32, tag="pk")
                    for c in range(NC_CH):
                        nc.tensor.matmul(
                            pq[lo:hi, :],
                            lhsT=_r(wq_sb[:, c, :]),
                            rhs=_r(xt[:, c, :]),
                            start=(c == 0),
                            stop=(c == NC_CH - 1),
                            tile_position=tp,
                        )
                    for c in range(NC_CH):
                        nc.tensor.matmul(
                            pk[lo:hi, :],
                            lhsT=_r(wk_sb[:, c, :]),
                            rhs=_r(xt[:, c, :]),
                            start=(c == 0),
                            stop=(c == NC_CH - 1),
                            tile_position=tp,
                        )
                    nc.vector.tensor_copy(
                        out=qT[lo:hi, j * 512 : (j + 1) * 512], in_=pq[lo:hi, :]
                    )
                    nc.vector.tensor_copy(
                        out=kT[lo:hi, j * 512 : (j + 1) * 512], in_=pk[lo:hi, :]
                    )
                    # v in natural [T, D] layout: x-tile chunks as stationary operand
                    for rr in range(4):
                        pv = psv.tile([128, D], f32, tag="pv")
                        for c in range(NC_CH):
                            nc.tensor.matmul(
                                pv[:],
                                lhsT=_r(xt[:, c, rr * 128 : (rr + 1) * 128]),
                                rhs=_r(wv_sb[:, c, :]),
                                start=(c == 0),
                                stop=(c == NC_CH - 1),
                            )
                        rt = b * NKT + j * 4 + rr
                        nc.vector.tensor_copy(out=v_sb[:, rt, 0:D], in_=pv[:])

        # ---------------- Phase 2: causal attention ----------------
        with (
            tc.tile_pool(name="p2p", bufs=3) as ppool,
            tc.tile_pool(name="p2s", bufs=2, space="PSUM") as pss,
            tc.tile_pool(name="p2y", bufs=2, space="PSUM") as psy,
        ):
            for j in range(NQT):
                q0 = j * 512
                nkt = 4 * (j + 1)  # causal k-tiles for this q block
                groups = [
                    list(range(s, min(s + KGRP, nkt))) for s in range(0, nkt, KGRP)
                ]
                yps = [
                    psy.tile([D + 1, 512], f32, tag="y", name=f"y_{j}_{b}")
                    for b in range(B)
                ]
                for g in groups:
                    for b in range(B):
                        lo, hi = 64 * b, 64 * b + 64
                        s4 = pss.tile([128, KGRP, 512], f32, tag="s")
                        for ui, kt in enumerate(g):
                            nc.tensor.matmul(
                                s4[:, ui, :],
                                lhsT=_r(kT[lo:hi, kt * 128 : (kt + 1) * 128]),
                                rhs=_r(qT[lo:hi, q0 : q0 + 512]),
                                start=True,
                                stop=True,
                            )
                        nu = len(g)
                        p4 = ppool.tile([128, KGRP, 512], f32r, tag="p")
                        # exp(s/sqrt(D)); scores are O(1) so no max subtraction
                        nc.scalar.activation(
                            out=p4[:, 0:nu, :],
                            in_=s4[:, 0:nu, :],
                            func=mybir.ActivationFunctionType.Exp,
                            scale=0.125,
                        )
                        for ui, kt in enumerate(g):
                            dlt = kt * 128 - q0
                            if dlt > -128:
                                # keep where (q0+col) >= (kt*128+p)
                                nc.gpsimd.affine_select(
                                    out=p4[:, ui, :],
                                    in_=p4[:, ui, :],
                                    compare_op=mybir.AluOpType.is_ge,
                                    fill=0.0,
                                    base=-dlt,
                                    channel_multiplier=-1,
                                    pattern=[[1, 512]],
                                )
                        for ui, kt in enumerate(g):
                            nc.tensor.matmul(
                                yps[b][:],
                                lhsT=_r(v_sb[:, b * NKT + kt, :]),
                                rhs=_r(p4[:, ui, :]),
                                start=(kt == 0),
                                stop=(kt == nkt - 1),
                            )
                for b in range(B):
                    nc.vector.tensor_copy(
                        out=yT[:, b * T + q0 : b * T + q0 + 512], in_=yps[b][:]
                    )

        # ---------------- Phase 3: c_proj partial + normalization ----------------
        with (
            tc.tile_pool(name="p3o", bufs=3) as opool,
            tc.tile_pool(name="p3ps", bufs=2, space="PSUM") as pso,
        ):
            for r in range(BT // 128):
                lhsT = yT[:, r * 128 : (r + 1) * 128]  # [65, 128]
                po = pso.tile([128, C], f32, tag="po")
                pu = pso.tile([128, 2], f32, tag="pu")
                nc.tensor.matmul(po[:], lhsT=_r(lhsT), rhs=_r(wp_sb[:]), start=True, stop=True)
                nc.tensor.matmul(pu[:], lhsT=_r(lhsT), rhs=_r(e_sb[:]), start=True, stop=True)
                recip = opool.tile([128, 1], f32, tag="recip")
                nc.vector.reciprocal(recip[:], pu[:, 0:1])
                ot = opool.tile([128, C], f32, tag="ot")
                nc.vector.tensor_scalar_mul(ot[:], in0=po[:], scalar1=recip[:])
                nc.sync.dma_start(outp[r * 128 : (r + 1) * 128, :], ot[:])

    nc.compile()
    return nc


_cache: dict = {}


def _get_nc() -> bass.Bass:
    if "nc" not in _cache:
        _cache["nc"] = build_kernel()
    return _cache["nc"]


def make_in_maps(x, W_attn, W_proj):
    import ml_dtypes
    xTq = np.ascontiguousarray(x.reshape(BT, C).T).astype(ml_dtypes.bfloat16)
    in_maps = []
    for i in range(NCORES):
        wp_pad = np.zeros((D + 1, C), dtype=np.float32)
        wp_pad[:D] = W_proj[i * D : (i + 1) * D, :]
        ev = np.zeros((D + 1, 2), dtype=np.float32)
        ev[D, 0] = 1.0
        in_maps.append(
            {
                "xT": xTq,
                "ev": ev,
                "ones64": np.ones(64, dtype=np.float32),
                "wq": np.ascontiguousarray(W_attn[:, i * D : (i + 1) * D]).astype(ml_dtypes.bfloat16),
                "wk": np.ascontiguousarray(W_attn[:, C + i * D : C + (i + 1) * D]).astype(ml_dtypes.bfloat16),
                "wv": np.ascontiguousarray(
                    W_attn[:, 2 * C + i * D : 2 * C + (i + 1) * D]
                ).astype(ml_dtypes.bfloat16),
                "wp": wp_pad,
            }
        )
    return in_maps


def kernel(x, W_attn, W_proj, _trace=False):
    from concourse.bass_utils import run_bass_kernel_spmd

    nc = _get_nc()
    in_maps = make_in_maps(
        np.asarray(x, dtype=np.float32),
        np.asarray(W_attn, dtype=np.float32),
        np.asarray(W_proj, dtype=np.float32),
    )
    res = run_bass_kernel_spmd(
        nc, in_maps, core_ids=list(range(NCORES)), trace=_trace
    )
    out = np.zeros((BT, C), dtype=np.float32)
    for r in res.results:
        out += r["outp"]
    out = out.reshape(B, T, C)
    if _trace:
        return out, res
    return out



# revision 2
# speedup vs baseline: 1.0993x; 1.0993x over previous
"""Causal self-attention Trainium2 kernel — fused pipeline version.

y = softmax_causal((x@Wq)(x@Wk)^T / sqrt(D)) @ (x@Wv) @ Wp

Sharding: head-parallel over 8 cores (H=8 heads, one per core), both batches
on every core (batch b occupies SBUF partitions 64b:64b+64 of qT/kT).
Each core produces its head's partial contribution to y @ W_proj; the host
sums the 8 partials.

Single fused software pipeline over 16 q-tiles of 256 positions. The scalar
engine's exp stream is the bottleneck; everything else (QKV projection,
c_proj, output DMA) is emitted as small "filler" chunks at attention-group
boundaries so the in-order PE stream never blocks on a cold dependency.

PSUM (8 banks): 2x [128,4,256] score slots (4) + 2 y slots (2) +
2 aux slots (2) shared by QKV projection and c_proj tiles.
"""

import sys

sys.path.insert(0, "/opt/trn_rl_repo")

from contextlib import ExitStack

import numpy as np

import concourse.bass as bass
import concourse.mybir as mybir
import concourse.tile as tile
from concourse import bacc

B, T, C, H, D = 2, 4096, 512, 8, 64
BT = B * T  # 8192
NCORES = 8
NC_CH = C // 128  # 4 contraction chunks for the QKV projection
QT = 256  # q-tile size
NQT = T // QT  # 16 q-tiles per batch
KGRP = 4  # k-tiles per exp group (fills one 2-bank PSUM slot)
NTK = T // 128  # 32 k-tiles per batch

f32 = mybir.dt.float32
bf16 = mybir.dt.bfloat16
fp8 = mybir.dt.float8e4


def build_kernel() -> bass.Bass:
    nc = bacc.Bacc()

    xT = nc.dram_tensor("xT", [C, BT], bf16, kind="ExternalInput")
    wq = nc.dram_tensor("wq", [C, D], bf16, kind="ExternalInput")
    wk = nc.dram_tensor("wk", [C, D], bf16, kind="ExternalInput")
    wv = nc.dram_tensor("wv", [C, D], bf16, kind="ExternalInput")
    wp = nc.dram_tensor("wp", [D, C], bf16, kind="ExternalInput")
    outp = nc.dram_tensor("outp", [BT, C], f32, kind="ExternalOutput")

    # [p, a, t]: row (a*128+p) of xT -> partition p, chunk a
    xTp = xT[:, :].rearrange("(a p) t -> p a t", p=128)

    with tile.TileContext(nc) as tc, ExitStack() as ctx:
        singles = ctx.enter_context(tc.tile_pool(name="singles", bufs=1))

        # Persistent SBUF tensors
        qT = singles.tile([128, T], bf16)  # [0:64]=batch0 head dims, [64:128]=b1
        kT = singles.tile([128, T], bf16)
        v_sb = singles.tile([128, B * NTK, D + 1], bf16)  # + ones column
        wq_sb = singles.tile([128, NC_CH, D], bf16)
        wk_sb = singles.tile([128, NC_CH, D], bf16)
        wv_sb = singles.tile([128, NC_CH, D], bf16)
        # W_proj duplicated in both partition halves so c_proj's rhs base
        # partition can match either yT chunk (HW: fmap/weight same base)
        wp_sb = singles.tile([128, C], bf16)

        nc.sync.dma_start(wq_sb[:], wq[:, :].rearrange("(a p) d -> p a d", p=128))
        nc.sync.dma_start(wk_sb[:], wk[:, :].rearrange("(a p) d -> p a d", p=128))
        nc.sync.dma_start(wv_sb[:], wv[:, :].rearrange("(a p) d -> p a d", p=128))
        nc.sync.dma_start(wp_sb[0:D, :], wp[:, :])
        nc.sync.dma_start(wp_sb[D:128, :], wp[:, :])
        nc.gpsimd.memset(v_sb[:, :, D], 1.0)

        ps = ctx.enter_context(tc.tile_pool(name="ps", bufs=2, space="PSUM"))
        ypool = ctx.enter_context(tc.tile_pool(name="yps", bufs=2, space="PSUM"))
        aux = ctx.enter_context(tc.tile_pool(name="aux", bufs=2, space="PSUM"))
        xpool = ctx.enter_context(tc.tile_pool(name="xt", bufs=4))
        ppool = ctx.enter_context(tc.tile_pool(name="p4", bufs=3))
        ysbp = ctx.enter_context(tc.tile_pool(name="ysb", bufs=3))
        rpool = ctx.enter_context(tc.tile_pool(name="rec", bufs=3))
        tpool = ctx.enter_context(tc.tile_pool(name="yT", bufs=3))
        opool = ctx.enter_context(tc.tile_pool(name="ot", bufs=3))

        # Deferred emission closures, popped at attention-group boundaries.
        # qkv fillers have a hard deadline (drained before the q-tile that
        # reads them); proj fillers drift freely behind.
        qkv_fillers: list = []
        proj_fillers: list = []

        def pop_fillers():
            if qkv_fillers:
                qkv_fillers.pop(0)()
            if proj_fillers:
                proj_fillers.pop(0)()

        def drain(fl):
            while fl:
                fl.pop(0)()

        xts: dict = {}

        def emit_x_load(J):
            """DMA the x block for t-range [512J, 512J+512), both batches."""
            tiles = []
            for b in range(B):
                xt = xpool.tile([128, NC_CH, 512], bf16, tag="xt")
                t0 = b * T + J * 512
                nc.gpsimd.dma_start(out=xt[:], in_=xTp[:, :, t0 : t0 + 512])
                tiles.append(xt)
            xts[J] = tiles

        def make_qk_filler(J, which):
            """Project q or k for t-block J in cheap [t, d] layout (ap-64
            matmuls), then DMA-transpose each 128-t chunk into the [d, t]
            qT/kT layout (batch in the partition halves)."""
            w_sb, dst = (wq_sb, qT) if which == "q" else (wk_sb, kT)

            def emit():
                # [t_lo, rr, b, d] so each rr chunk transposes to [b*64+d, t]
                ptd = aux.tile([128, 4, B, D], f32, tag="aux", name=f"p{which}_{J}")
                for rr in range(4):
                    for b in range(B):
                        for c in range(NC_CH):
                            nc.tensor.matmul(
                                ptd[:, rr, b, :],
                                lhsT=xts[J][b][:, c, rr * 128 : (rr + 1) * 128],
                                rhs=w_sb[:, c, :],
                                start=(c == 0),
                                stop=(c == NC_CH - 1),
                            )
                td_sb = ysbp.tile([128, 4, B, D], bf16, tag="td")
                nc.vector.tensor_copy(out=td_sb[:], in_=ptd[:])
                for rr in range(4):
                    t0 = J * 512 + rr * 128
                    nc.sync.dma_start_transpose(
                        dst[:, t0 : t0 + 128], td_sb[:, rr, :, :]
                    )

            return emit

        def make_v_filler(J):
            def emit():
                pv = aux.tile([128, 512], f32, tag="aux", name=f"pv_{J}")
                for b in range(B):
                    for rr in range(4):
                        sl = pv[:, (b * 4 + rr) * 64 : (b * 4 + rr + 1) * 64]
                        for c in range(NC_CH):
                            nc.tensor.matmul(
                                sl,
                                lhsT=xts[J][b][:, c, rr * 128 : (rr + 1) * 128],
                                rhs=wv_sb[:, c, :],
                                start=(c == 0),
                                stop=(c == NC_CH - 1),
                            )
                for b in range(B):
                    a0 = b * NTK + J * 4
                    nc.vector.tensor_copy(
                        out=v_sb[:, a0 : a0 + 4, 0:D],
                        in_=pv[:, b * 256 : (b + 1) * 256],
                    )

            return emit

        def make_proj_filler(jq, b, qc, yTt, rec):
            """c_proj + normalize + store for one 128-row output chunk."""

            def emit():
                po = aux.tile([128, C], f32, tag="aux", name=f"po_{jq}_{b}_{qc}")
                nc.tensor.matmul(
                    po[:],
                    lhsT=yTt[qc * 64 : (qc + 1) * 64, :],
                    rhs=wp_sb[qc * 64 : (qc + 1) * 64, :],
                    start=True,
                    stop=True,
                )
                ot = opool.tile([128, C], f32, tag="ot")
                nc.vector.tensor_scalar_mul(
                    ot[:], in0=po[:], scalar1=rec[:, qc : qc + 1]
                )
                r0 = b * T + jq * QT + qc * 128
                nc.sync.dma_start(outp[r0 : r0 + 128, :], ot[:])

            return emit

        def emit_attention(jq, b, y):
            """Causal attention for q-tile jq (256 q), batch b -> y PSUM."""
            q0 = jq * QT
            nkt = 2 * (jq + 1)
            lo, hi = 64 * b, 64 * b + 64
            groups = [
                list(range(s, min(s + KGRP, nkt))) for s in range(0, nkt, KGRP)
            ]
            pending = None  # (group, p4 tile) awaiting PV emission

            def emit_pv(g, p):
                # Both qc chunks live in one PSUM bank (= one zero region):
                # a single start arms the whole bank, a single stop closes it.
                for ui, kt in enumerate(g):
                    for qc in range(2):
                        nc.tensor.matmul(
                            y[:, qc, :],
                            lhsT=p[:, ui, qc * 128 : (qc + 1) * 128],
                            rhs=v_sb[:, b * NTK + kt, :],
                            start=(kt == 0 and qc == 0),
                            stop=(kt == nkt - 1 and qc == 1),
                        )

            for g in groups:
                s = ps.tile([128, KGRP, QT], f32, tag="s", name=f"s_{jq}_{b}_{g[0]}")
                for ui, kt in enumerate(g):
                    nc.tensor.matmul(
                        s[:, ui, :],
                        lhsT=kT[lo:hi, kt * 128 : (kt + 1) * 128],
                        rhs=qT[lo:hi, q0 : q0 + QT],
                        start=True,
                        stop=True,
                    )
                nu = len(g)
                p = ppool.tile([128, KGRP, QT], bf16, tag="p")
                # exp(s/sqrt(D)); scores are O(1) so no max subtraction
                nc.scalar.activation(
                    out=p[:, 0:nu, :],
                    in_=s[:, 0:nu, :],
                    func=mybir.ActivationFunctionType.Exp,
                    scale=0.125,
                )
                for ui, kt in enumerate(g):
                    dlt = kt * 128 - q0
                    if dlt >= 0:
                        w = dlt + 128  # cols >= w are all-keep
                        nc.gpsimd.affine_select(
                            out=p[:, ui, 0:w],
                            in_=p[:, ui, 0:w],
                            compare_op=mybir.AluOpType.is_ge,
                            fill=0.0,
                            base=-dlt,
                            channel_multiplier=-1,
                            pattern=[[1, w]],
                        )
                if pending is not None:
                    emit_pv(*pending)
                pending = (g, p)
                pop_fillers()
            emit_pv(*pending)

        def emit_norm(jq, b, y):
            rec = rpool.tile([128, 2], f32, tag="rec")
            nc.vector.reciprocal(rec[:], y[:, :, D])
            y_sb = ysbp.tile([128, 2, D], bf16, tag="ysb")
            nc.vector.tensor_copy(out=y_sb[:], in_=y[:, :, 0:D])
            yTt = tpool.tile([128, 128], bf16, tag="yT")
            nc.sync.dma_start_transpose(yTt[:], y_sb[:, :, :])
            for qc in range(2):
                proj_fillers.append(make_proj_filler(jq, b, qc, yTt, rec))

        # ---------------- prologue: first x block + qkv(0) ----------------
        emit_x_load(0)
        make_qk_filler(0, "q")()
        make_qk_filler(0, "k")()
        make_v_filler(0)()

        for jq in range(NQT):
            if jq % 2 == 0:
                drain(qkv_fillers)  # this q-tile reads the staged block
            else:
                if jq < NQT - 1:
                    # stage next 512-t block: DMA now, project via fillers
                    J = (jq + 1) // 2
                    emit_x_load(J)
                    qkv_fillers.append(make_qk_filler(J, "q"))
                    qkv_fillers.append(make_qk_filler(J, "k"))
                    qkv_fillers.append(make_v_filler(J))
            for b in range(B):
                y = ypool.tile([128, 2, D + 1], f32, tag="y", name=f"y_{jq}_{b}")
                emit_attention(jq, b, y)
                emit_norm(jq, b, y)

        drain(proj_fillers)

    nc.compile()
    return nc


_cache: dict = {}


def _get_nc() -> bass.Bass:
    if "nc" not in _cache:
        _cache["nc"] = build_kernel()
    return _cache["nc"]


def make_in_maps(x, W_attn, W_proj):
    import ml_dtypes

    xTq = np.ascontiguousarray(x.reshape(BT, C).T).astype(ml_dtypes.bfloat16)
    in_maps = []
    for i in range(NCORES):
        in_maps.append(
            {
                "xT": xTq,
                "wq": np.ascontiguousarray(W_attn[:, i * D : (i + 1) * D]).astype(
                    ml_dtypes.bfloat16
                ),
                "wk": np.ascontiguousarray(
                    W_attn[:, C + i * D : C + (i + 1) * D]
                ).astype(ml_dtypes.bfloat16),
                "wv": np.ascontiguousarray(
                    W_attn[:, 2 * C + i * D : 2 * C + (i + 1) * D]
                ).astype(ml_dtypes.bfloat16),
                "wp": np.ascontiguousarray(W_proj[i * D : (i + 1) * D, :]).astype(
                    ml_dtypes.bfloat16
                ),
            }
        )
    return in_maps


def kernel(x, W_attn, W_proj, _trace=False):
    from concourse.bass_utils import run_bass_kernel_spmd

    nc = _get_nc()
    in_maps = make_in_maps(
        np.asarray(x, dtype=np.float32),
        np.asarray(W_attn, dtype=np.float32),
        np.asarray(W_proj, dtype=np.float32),
    )
    res = run_bass_kernel_spmd(
        nc, in_maps, core_ids=list(range(NCORES)), trace=_trace
    )
    out = np.zeros((BT, C), dtype=np.float32)
    for r in res.results:
        out += r["outp"]
    out = out.reshape(B, T, C)
    if _trace:
        return out, res
    return out


# revision 3
# speedup vs baseline: 1.1121x; 1.0116x over previous
"""Causal self-attention Trainium2 kernel — fused pipeline version.

y = softmax_causal((x@Wq)(x@Wk)^T / sqrt(D)) @ (x@Wv) @ Wp

Sharding: head-parallel over 8 cores (H=8 heads, one per core), both batches
on every core (batch b occupies SBUF partitions 64b:64b+64 of qT/kT).
Each core produces its head's partial contribution to y @ W_proj; the host
sums the 8 partials.

Single fused software pipeline over 16 q-tiles of 256 positions. The scalar
engine's exp stream is the bottleneck; everything else (QKV projection,
c_proj, output DMA) is emitted as small "filler" chunks at attention-group
boundaries so the in-order PE stream never blocks on a cold dependency.

PSUM (8 banks): 2x [128,4,256] score slots (4) + 2 y slots (2) +
2 aux slots (2) shared by QKV projection and c_proj tiles.
"""

import sys

sys.path.insert(0, "/opt/trn_rl_repo")

from contextlib import ExitStack

import numpy as np

import concourse.bass as bass
import concourse.mybir as mybir
import concourse.tile as tile
from concourse import bacc

B, T, C, H, D = 2, 4096, 512, 8, 64
BT = B * T  # 8192
NCORES = 8
NC_CH = C // 128  # 4 contraction chunks for the QKV projection
QT = 256  # q-tile size
NQT = T // QT  # 16 q-tiles per batch
KGRP = 4  # k-tiles per exp group (fills one 2-bank PSUM slot)
NTK = T // 128  # 32 k-tiles per batch

f32 = mybir.dt.float32
bf16 = mybir.dt.bfloat16
fp8 = mybir.dt.float8e4


def build_kernel() -> bass.Bass:
    nc = bacc.Bacc()

    xT = nc.dram_tensor("xT", [C, BT], bf16, kind="ExternalInput")
    wqkv = nc.dram_tensor("wqkv", [C, 3 * D], bf16, kind="ExternalInput")
    wp = nc.dram_tensor("wp", [D, C], bf16, kind="ExternalInput")
    outp = nc.dram_tensor("outp", [BT, C], f32, kind="ExternalOutput")

    # [p, a, t]: row (a*128+p) of xT -> partition p, chunk a
    xTp = xT[:, :].rearrange("(a p) t -> p a t", p=128)

    with tile.TileContext(nc) as tc, ExitStack() as ctx:
        singles = ctx.enter_context(tc.tile_pool(name="singles", bufs=1))

        # Persistent SBUF tensors
        qT = singles.tile([128, T], bf16)  # [0:64]=batch0 head dims, [64:128]=b1
        kT = singles.tile([128, T], bf16)
        v_sb = singles.tile([128, B * NTK, D + 1], bf16)  # + ones column
        w_sb = singles.tile([128, NC_CH, 3 * D], bf16)  # [q | k | v] slices
        # W_proj duplicated in both partition halves so c_proj's rhs base
        # partition can match either yT chunk (HW: fmap/weight same base)
        wp_sb = singles.tile([128, C], bf16)

        nc.gpsimd.memset(v_sb[:, :, D], 1.0)

        ps = ctx.enter_context(tc.tile_pool(name="ps", bufs=2, space="PSUM"))
        ypool = ctx.enter_context(tc.tile_pool(name="yps", bufs=2, space="PSUM"))
        aux = ctx.enter_context(tc.tile_pool(name="aux", bufs=2, space="PSUM"))
        xpool = ctx.enter_context(tc.tile_pool(name="xt", bufs=8))
        ppool = ctx.enter_context(tc.tile_pool(name="p4", bufs=4))
        ysbp = ctx.enter_context(tc.tile_pool(name="ysb", bufs=3))
        rpool = ctx.enter_context(tc.tile_pool(name="rec", bufs=3))
        tpool = ctx.enter_context(tc.tile_pool(name="yT", bufs=3))
        opool = ctx.enter_context(tc.tile_pool(name="ot", bufs=3))

        # Deferred emission closures, popped at attention-group boundaries.
        # qkv fillers have a hard deadline (drained before the q-tile that
        # reads them); proj fillers drift freely behind.
        qkv_fillers: list = []
        proj_fillers: list = []

        def pop_fillers():
            for _ in range(2):
                if qkv_fillers:
                    qkv_fillers.pop(0)()
                if proj_fillers:
                    proj_fillers.pop(0)()

        def drain(fl):
            while fl:
                fl.pop(0)()

        xts: dict = {}

        def emit_x_load(J):
            """DMA the x block for t-range [512J, 512J+512), both batches."""
            tiles = []
            for b in range(B):
                xt = xpool.tile([128, NC_CH, 512], bf16, tag="xt")
                t0 = b * T + J * 512
                nc.sync.dma_start(xt[:], xTp[:, :, t0 : t0 + 512])
                tiles.append(xt)
            xts[J] = tiles

        def make_qk_filler(J, which):
            """Project q or k for t-block J in cheap [t, d] layout (ap-64
            matmuls), then DMA-transpose each 128-t chunk into the [d, t]
            qT/kT layout (batch in the partition halves)."""
            w0, dst = (0, qT) if which == "q" else (D, kT)

            def emit():
                # [t_lo, rr, b, d] so each rr chunk transposes to [b*64+d, t]
                ptd = aux.tile([128, 4, B, D], f32, tag="aux", name=f"p{which}_{J}")
                for rr in range(4):
                    for b in range(B):
                        for c in range(NC_CH):
                            nc.tensor.matmul(
                                ptd[:, rr, b, :],
                                lhsT=xts[J][b][:, c, rr * 128 : (rr + 1) * 128],
                                rhs=w_sb[:, c, w0 : w0 + D],
                                start=(c == 0),
                                stop=(c == NC_CH - 1),
                            )
                td_sb = ysbp.tile([128, 4, B, D], bf16, tag="td")
                for rr in range(4):
                    nc.vector.tensor_copy(
                        out=td_sb[:, rr, :, :], in_=ptd[:, rr, :, :]
                    )
                    t0 = J * 512 + rr * 128
                    nc.sync.dma_start_transpose(
                        dst[:, t0 : t0 + 128], td_sb[:, rr, :, :]
                    )

            return emit

        def make_v_filler(J):
            def emit():
                pv = aux.tile([128, 512], f32, tag="aux", name=f"pv_{J}")
                for b in range(B):
                    for rr in range(4):
                        sl = pv[:, (b * 4 + rr) * 64 : (b * 4 + rr + 1) * 64]
                        for c in range(NC_CH):
                            nc.tensor.matmul(
                                sl,
                                lhsT=xts[J][b][:, c, rr * 128 : (rr + 1) * 128],
                                rhs=w_sb[:, c, 2 * D : 3 * D],
                                start=(c == 0),
                                stop=(c == NC_CH - 1),
                            )
                for b in range(B):
                    a0 = b * NTK + J * 4
                    nc.vector.tensor_copy(
                        out=v_sb[:, a0 : a0 + 4, 0:D],
                        in_=pv[:, b * 256 : (b + 1) * 256],
                    )

            return emit

        def make_proj_filler(jq, b, qc, yTt, rec):
            """c_proj + normalize + store for one 128-row output chunk."""

            def emit():
                po = aux.tile([128, C], f32, tag="aux", name=f"po_{jq}_{b}_{qc}")
                nc.tensor.matmul(
                    po[:],
                    lhsT=yTt[qc * 64 : (qc + 1) * 64, :],
                    rhs=wp_sb[qc * 64 : (qc + 1) * 64, :],
                    start=True,
                    stop=True,
                )
                ot = opool.tile([128, C], f32, tag="ot")
                nc.vector.tensor_scalar_mul(
                    ot[:], in0=po[:], scalar1=rec[:, qc : qc + 1]
                )
                r0 = b * T + jq * QT + qc * 128
                nc.sync.dma_start(outp[r0 : r0 + 128, :], ot[:])

            return emit

        def emit_attention(jq, b, y):
            """Causal attention for q-tile jq (256 q), batch b -> y PSUM."""
            q0 = jq * QT
            nkt = 2 * (jq + 1)
            lo, hi = 64 * b, 64 * b + 64
            groups = [
                list(range(s, min(s + KGRP, nkt))) for s in range(0, nkt, KGRP)
            ]
            pending = None  # (group, p4 tile) awaiting PV emission

            def emit_pv(g, p):
                # Both qc chunks live in one PSUM bank (= one zero region):
                # a single start arms the whole bank, a single stop closes it.
                for ui, kt in enumerate(g):
                    for qc in range(2):
                        nc.tensor.matmul(
                            y[:, qc, :],
                            lhsT=p[:, ui, qc * 128 : (qc + 1) * 128],
                            rhs=v_sb[:, b * NTK + kt, :],
                            start=(kt == 0 and qc == 0),
                            stop=(kt == nkt - 1 and qc == 1),
                        )

            for g in groups:
                s = ps.tile([128, KGRP, QT], f32, tag="s", name=f"s_{jq}_{b}_{g[0]}")
                for ui, kt in enumerate(g):
                    nc.tensor.matmul(
                        s[:, ui, :],
                        lhsT=kT[lo:hi, kt * 128 : (kt + 1) * 128],
                        rhs=qT[lo:hi, q0 : q0 + QT],
                        start=True,
                        stop=True,
                    )
                nu = len(g)
                p = ppool.tile([128, KGRP, QT], bf16, tag="p")
                # exp(s/sqrt(D)); scores are O(1) so no max subtraction
                nc.scalar.activation(
                    out=p[:, 0:nu, :],
                    in_=s[:, 0:nu, :],
                    func=mybir.ActivationFunctionType.Exp,
                    scale=0.125,
                )
                for ui, kt in enumerate(g):
                    dlt = kt * 128 - q0
                    if dlt >= 0:
                        w = dlt + 128  # cols >= w are all-keep
                        nc.gpsimd.affine_select(
                            out=p[:, ui, 0:w],
                            in_=p[:, ui, 0:w],
                            compare_op=mybir.AluOpType.is_ge,
                            fill=0.0,
                            base=-dlt,
                            channel_multiplier=-1,
                            pattern=[[1, w]],
                        )
                if pending is not None:
                    emit_pv(*pending)
                pending = (g, p)
                pop_fillers()
            emit_pv(*pending)

        def emit_norm(jq, b, y):
            rec = rpool.tile([128, 2], f32, tag="rec")
            nc.vector.reciprocal(rec[:], y[:, :, D])
            y_sb = ysbp.tile([128, 2, D], bf16, tag="ysb")
            nc.vector.tensor_copy(out=y_sb[:], in_=y[:, :, 0:D])
            yTt = tpool.tile([128, 128], bf16, tag="yT")
            nc.sync.dma_start_transpose(yTt[:], y_sb[:, :, :])
            for qc in range(2):
                proj_fillers.append(make_proj_filler(jq, b, qc, yTt, rec))

        # ------------- prologue: first x blocks, qkv(0)+qkv(1) direct -------------
        emit_x_load(0)
        nc.sync.dma_start(w_sb[:], wqkv[:, :].rearrange("(a p) d -> p a d", p=128))
        # pre-warm the exp table while ACT is otherwise idle
        warm = rpool.tile([128, 1], f32, tag="warm")
        nc.scalar.activation(
            out=warm[:], in_=v_sb[:, 0:1, D],
            func=mybir.ActivationFunctionType.Exp,
        )
        make_qk_filler(0, "q")()
        make_qk_filler(0, "k")()
        emit_x_load(1)
        make_v_filler(0)()
        make_qk_filler(1, "q")()
        make_qk_filler(1, "k")()
        make_v_filler(1)()
        emit_x_load(2)
        nc.sync.dma_start(wp_sb[0:D, :], wp[:, :])
        nc.sync.dma_start(wp_sb[D:128, :], wp[:, :])
        qkv_fillers.append(make_qk_filler(2, "q"))
        qkv_fillers.append(make_qk_filler(2, "k"))
        qkv_fillers.append(make_v_filler(2))

        for jq in range(NQT):
            if jq % 2 == 0:
                drain(qkv_fillers)  # this q-tile reads the staged block
            else:
                J = (jq + 5) // 2
                if J < NQT // 2:
                    # stage a 512-t block one tile ahead: DMA now, project
                    # via fillers popped at group boundaries
                    emit_x_load(J)
                    qkv_fillers.append(make_qk_filler(J, "q"))
                    qkv_fillers.append(make_qk_filler(J, "k"))
                    qkv_fillers.append(make_v_filler(J))
            for b in range(B):
                y = ypool.tile([128, 2, D + 1], f32, tag="y", name=f"y_{jq}_{b}")
                emit_attention(jq, b, y)
                emit_norm(jq, b, y)

        drain(proj_fillers)

    nc.compile()
    return nc


_cache: dict = {}


def _get_nc() -> bass.Bass:
    if "nc" not in _cache:
        _cache["nc"] = build_kernel()
    return _cache["nc"]


def make_in_maps(x, W_attn, W_proj):
    import ml_dtypes

    xTq = np.ascontiguousarray(x.reshape(BT, C).T).astype(ml_dtypes.bfloat16)
    in_maps = []
    for i in range(NCORES):
        wqkv = np.concatenate(
            [
                W_attn[:, i * D : (i + 1) * D],
                W_attn[:, C + i * D : C + (i + 1) * D],
                W_attn[:, 2 * C + i * D : 2 * C + (i + 1) * D],
            ],
            axis=1,
        )
        in_maps.append(
            {
                "xT": xTq,
                "wqkv": np.ascontiguousarray(wqkv).astype(ml_dtypes.bfloat16),
                "wp": np.ascontiguousarray(W_proj[i * D : (i + 1) * D, :]).astype(
                    ml_dtypes.bfloat16
                ),
            }
        )
    return in_maps


def kernel(x, W_attn, W_proj, _trace=False):
    from concourse.bass_utils import run_bass_kernel_spmd

    nc = _get_nc()
    in_maps = make_in_maps(
        np.asarray(x, dtype=np.float32),
        np.asarray(W_attn, dtype=np.float32),
        np.asarray(W_proj, dtype=np.float32),
    )
    res = run_bass_kernel_spmd(
        nc, in_maps, core_ids=list(range(NCORES)), trace=_trace
    )
    out = np.zeros((BT, C), dtype=np.float32)
    for r in res.results:
        out += r["outp"]
    out = out.reshape(B, T, C)
    if _trace:
        return out, res
    return out


# revision 4
# speedup vs baseline: 1.1199x; 1.0070x over previous
"""Causal self-attention Trainium2 kernel — fused pipeline version.

y = softmax_causal((x@Wq)(x@Wk)^T / sqrt(D)) @ (x@Wv) @ Wp

Sharding: head-parallel over 8 cores (H=8 heads, one per core), both batches
on every core (batch b occupies SBUF partitions 64b:64b+64 of qT/kT).
Each core produces its head's partial contribution to y @ W_proj; the host
sums the 8 partials.

Single fused software pipeline over 16 q-tiles of 256 positions. The scalar
engine's exp stream is the bottleneck; everything else (QKV projection,
c_proj, output DMA) is emitted as small "filler" chunks at attention-group
boundaries so the in-order PE stream never blocks on a cold dependency.

PSUM (8 banks): 2x [128,4,256] score slots (4) + 2 y slots (2) +
2 aux slots (2) shared by QKV projection and c_proj tiles.
"""

import sys

sys.path.insert(0, "/opt/trn_rl_repo")

from contextlib import ExitStack

import numpy as np

import concourse.bass as bass
import concourse.mybir as mybir
import concourse.tile as tile
from concourse import bacc

B, T, C, H, D = 2, 4096, 512, 8, 64
BT = B * T  # 8192
NCORES = 8
NC_CH = C // 128  # 4 contraction chunks for the QKV projection
QT = 256  # q-tile size
NQT = T // QT  # 16 q-tiles per batch
KGRP = 4  # k-tiles per exp group (fills one 2-bank PSUM slot)
NTK = T // 128  # 32 k-tiles per batch

f32 = mybir.dt.float32
bf16 = mybir.dt.bfloat16
fp8 = mybir.dt.float8e4


def build_kernel() -> bass.Bass:
    nc = bacc.Bacc()

    xT = nc.dram_tensor("xT", [C, BT], bf16, kind="ExternalInput")
    wqkv = nc.dram_tensor("wqkv", [C, 3 * D], bf16, kind="ExternalInput")
    wp = nc.dram_tensor("wp", [D, C], bf16, kind="ExternalInput")
    outp = nc.dram_tensor("outp", [BT, C], f32, kind="ExternalOutput")

    # [p, a, t]: row (a*128+p) of xT -> partition p, chunk a
    xTp = xT[:, :].rearrange("(a p) t -> p a t", p=128)

    with tile.TileContext(nc) as tc, ExitStack() as ctx:
        singles = ctx.enter_context(tc.tile_pool(name="singles", bufs=1))

        # Persistent SBUF tensors
        qT = singles.tile([128, T], bf16)  # [0:64]=batch0 head dims, [64:128]=b1
        kT = singles.tile([128, T], bf16)
        v_sb = singles.tile([128, B * NTK, D + 1], bf16)  # + ones column
        w_sb = singles.tile([128, NC_CH, 3 * D], bf16)  # [q | k | v] slices
        # W_proj duplicated in both partition halves so c_proj's rhs base
        # partition can match either yT chunk (HW: fmap/weight same base)
        wp_sb = singles.tile([128, C], bf16)

        nc.gpsimd.memset(v_sb[:, :, D], 1.0)

        ps = ctx.enter_context(tc.tile_pool(name="ps", bufs=2, space="PSUM"))
        ypool = ctx.enter_context(tc.tile_pool(name="yps", bufs=2, space="PSUM"))
        aux = ctx.enter_context(tc.tile_pool(name="aux", bufs=2, space="PSUM"))
        xpool = ctx.enter_context(tc.tile_pool(name="xt", bufs=8))
        ppool = ctx.enter_context(tc.tile_pool(name="p4", bufs=4))
        ysbp = ctx.enter_context(tc.tile_pool(name="ysb", bufs=3))
        rpool = ctx.enter_context(tc.tile_pool(name="rec", bufs=3))
        tpool = ctx.enter_context(tc.tile_pool(name="yT", bufs=3))
        opool = ctx.enter_context(tc.tile_pool(name="ot", bufs=3))

        # Deferred emission closures, popped at attention-group boundaries.
        # qkv fillers have a hard deadline (drained before the q-tile that
        # reads them); proj fillers drift freely behind.
        qkv_fillers: list = []
        proj_fillers: list = []

        def pop_fillers():
            for _ in range(2):
                if qkv_fillers:
                    qkv_fillers.pop(0)()
                if proj_fillers:
                    proj_fillers.pop(0)()

        def drain(fl):
            while fl:
                fl.pop(0)()

        xts: dict = {}

        def emit_x_load(J):
            """DMA the x block for t-range [512J, 512J+512), both batches."""
            tiles = []
            for b in range(B):
                xt = xpool.tile([128, NC_CH, 512], bf16, tag="xt")
                t0 = b * T + J * 512
                nc.sync.dma_start(xt[:], xTp[:, :, t0 : t0 + 512])
                tiles.append(xt)
            xts[J] = tiles

        def make_qk_fillers(J, which):
            """Project q or k for t-block J in cheap [t, d] layout (ap-64
            matmuls), then DMA-transpose each 128-t chunk into the [d, t]
            qT/kT layout (batch in the partition halves). One sub-filler per
            128-t chunk keeps PE lumps small."""
            w0, dst = (0, qT) if which == "q" else (D, kT)
            tiles = {}

            def emit_rr(rr):
                if not tiles:
                    # [t_lo, rr, b, d]: each rr chunk transposes to [b*64+d, t]
                    tiles["ptd"] = aux.tile(
                        [128, 4, B, D], f32, tag="aux", name=f"p{which}_{J}"
                    )
                    tiles["td"] = ysbp.tile(
                        [128, 4, B, D], bf16, tag="td", name=f"td{which}_{J}"
                    )
                ptd, td_sb = tiles["ptd"], tiles["td"]
                for b in range(B):
                    for c in range(NC_CH):
                        nc.tensor.matmul(
                            ptd[:, rr, b, :],
                            lhsT=xts[J][b][:, c, rr * 128 : (rr + 1) * 128],
                            rhs=w_sb[:, c, w0 : w0 + D],
                            start=(c == 0),
                            stop=(c == NC_CH - 1),
                        )
                nc.vector.tensor_copy(out=td_sb[:, rr, :, :], in_=ptd[:, rr, :, :])
                t0 = J * 512 + rr * 128
                nc.sync.dma_start_transpose(
                    dst[:, t0 : t0 + 128], td_sb[:, rr, :, :]
                )

            return [lambda rr=rr: emit_rr(rr) for rr in range(4)]

        def make_v_fillers(J):
            tiles = {}

            def emit_b(b):
                if not tiles:
                    tiles["pv"] = aux.tile([128, 512], f32, tag="aux", name=f"pv_{J}")
                pv = tiles["pv"]
                for rr in range(4):
                    sl = pv[:, (b * 4 + rr) * 64 : (b * 4 + rr + 1) * 64]
                    for c in range(NC_CH):
                        nc.tensor.matmul(
                            sl,
                            lhsT=xts[J][b][:, c, rr * 128 : (rr + 1) * 128],
                            rhs=w_sb[:, c, 2 * D : 3 * D],
                            start=(c == 0),
                            stop=(c == NC_CH - 1),
                        )
                a0 = b * NTK + J * 4
                nc.vector.tensor_copy(
                    out=v_sb[:, a0 : a0 + 4, 0:D],
                    in_=pv[:, b * 256 : (b + 1) * 256],
                )

            return [lambda b=b: emit_b(b) for b in range(B)]

        def make_proj_filler(jq, b, qc, yTt, rec):
            """c_proj + normalize + store for one 128-row output chunk."""

            def emit():
                po = aux.tile([128, C], f32, tag="aux", name=f"po_{jq}_{b}_{qc}")
                nc.tensor.matmul(
                    po[:],
                    lhsT=yTt[qc * 64 : (qc + 1) * 64, :],
                    rhs=wp_sb[qc * 64 : (qc + 1) * 64, :],
                    start=True,
                    stop=True,
                )
                ot = opool.tile([128, C], f32, tag="ot")
                nc.vector.tensor_scalar_mul(
                    ot[:], in0=po[:], scalar1=rec[:, qc : qc + 1]
                )
                r0 = b * T + jq * QT + qc * 128
                nc.sync.dma_start(outp[r0 : r0 + 128, :], ot[:])

            return emit

        def emit_attention(jq, b, y):
            """Causal attention for q-tile jq (256 q), batch b -> y PSUM."""
            q0 = jq * QT
            nkt = 2 * (jq + 1)
            lo, hi = 64 * b, 64 * b + 64
            groups = [
                list(range(s, min(s + KGRP, nkt))) for s in range(0, nkt, KGRP)
            ]
            pending = None  # (group, p4 tile) awaiting PV emission

            def emit_pv(g, p):
                # Both qc chunks live in one PSUM bank (= one zero region):
                # a single start arms the whole bank, a single stop closes it.
                for ui, kt in enumerate(g):
                    for qc in range(2):
                        nc.tensor.matmul(
                            y[:, qc, :],
                            lhsT=p[:, ui, qc * 128 : (qc + 1) * 128],
                            rhs=v_sb[:, b * NTK + kt, :],
                            start=(kt == 0 and qc == 0),
                            stop=(kt == nkt - 1 and qc == 1),
                        )

            for g in groups:
                s = ps.tile([128, KGRP, QT], f32, tag="s", name=f"s_{jq}_{b}_{g[0]}")
                for ui, kt in enumerate(g):
                    nc.tensor.matmul(
                        s[:, ui, :],
                        lhsT=kT[lo:hi, kt * 128 : (kt + 1) * 128],
                        rhs=qT[lo:hi, q0 : q0 + QT],
                        start=True,
                        stop=True,
                    )
                nu = len(g)
                p = ppool.tile([128, KGRP, QT], bf16, tag="p")
                # exp(s/sqrt(D)); scores are O(1) so no max subtraction
                nc.scalar.activation(
                    out=p[:, 0:nu, :],
                    in_=s[:, 0:nu, :],
                    func=mybir.ActivationFunctionType.Exp,
                    scale=0.125,
                )
                for ui, kt in enumerate(g):
                    dlt = kt * 128 - q0
                    if dlt >= 0:
                        w = dlt + 128  # cols >= w are all-keep
                        nc.gpsimd.affine_select(
                            out=p[:, ui, 0:w],
                            in_=p[:, ui, 0:w],
                            compare_op=mybir.AluOpType.is_ge,
                            fill=0.0,
                            base=-dlt,
                            channel_multiplier=-1,
                            pattern=[[1, w]],
                        )
                if pending is not None:
                    emit_pv(*pending)
                pending = (g, p)
                pop_fillers()
            emit_pv(*pending)

        def emit_norm(jq, b, y):
            rec = rpool.tile([128, 2], f32, tag="rec")
            nc.vector.reciprocal(rec[:], y[:, :, D])
            y_sb = ysbp.tile([128, 2, D], bf16, tag="ysb")
            nc.vector.tensor_copy(out=y_sb[:], in_=y[:, :, 0:D])
            yTt = tpool.tile([128, 128], bf16, tag="yT")
            nc.sync.dma_start_transpose(yTt[:], y_sb[:, :, :])
            for qc in range(2):
                proj_fillers.append(make_proj_filler(jq, b, qc, yTt, rec))

        # ------------- prologue: first x blocks, qkv(0)+qkv(1) direct -------------
        emit_x_load(0)
        nc.sync.dma_start(w_sb[:], wqkv[:, :].rearrange("(a p) d -> p a d", p=128))
        # pre-warm the exp table while ACT is otherwise idle
        warm = rpool.tile([128, 1], f32, tag="warm")
        nc.scalar.activation(
            out=warm[:], in_=v_sb[:, 0:1, D],
            func=mybir.ActivationFunctionType.Exp,
        )
        for fq, fk in zip(make_qk_fillers(0, "q"), make_qk_fillers(0, "k")):
            fq()
            fk()
        emit_x_load(1)
        for f in make_v_fillers(0):
            f()
        for f in make_qk_fillers(1, "q"):
            f()
        for f in make_qk_fillers(1, "k"):
            f()
        for f in make_v_fillers(1):
            f()
        emit_x_load(2)
        nc.sync.dma_start(wp_sb[0:D, :], wp[:, :])
        nc.sync.dma_start(wp_sb[D:128, :], wp[:, :])
        qkv_fillers.extend(make_qk_fillers(2, "q"))
        qkv_fillers.extend(make_qk_fillers(2, "k"))
        qkv_fillers.extend(make_v_fillers(2))

        for jq in range(NQT):
            if jq % 2 == 0:
                drain(qkv_fillers)  # this q-tile reads the staged block
            else:
                J = (jq + 5) // 2
                if J < NQT // 2:
                    # stage a 512-t block one tile ahead: DMA now, project
                    # via fillers popped at group boundaries
                    emit_x_load(J)
                    qkv_fillers.extend(make_qk_fillers(J, "q"))
                    qkv_fillers.extend(make_qk_fillers(J, "k"))
                    qkv_fillers.extend(make_v_fillers(J))
            for b in range(B):
                y = ypool.tile([128, 2, D + 1], f32, tag="y", name=f"y_{jq}_{b}")
                emit_attention(jq, b, y)
                emit_norm(jq, b, y)

        drain(proj_fillers)

    nc.compile()
    return nc


_cache: dict = {}


def _get_nc() -> bass.Bass:
    if "nc" not in _cache:
        _cache["nc"] = build_kernel()
    return _cache["nc"]


def make_in_maps(x, W_attn, W_proj):
    import ml_dtypes

    xTq = np.ascontiguousarray(x.reshape(BT, C).T).astype(ml_dtypes.bfloat16)
    in_maps = []
    for i in range(NCORES):
        wqkv = np.concatenate(
            [
                W_attn[:, i * D : (i + 1) * D],
                W_attn[:, C + i * D : C + (i + 1) * D],
                W_attn[:, 2 * C + i * D : 2 * C + (i + 1) * D],
            ],
            axis=1,
        )
        in_maps.append(
            {
                "xT": xTq,
                "wqkv": np.ascontiguousarray(wqkv).astype(ml_dtypes.bfloat16),
                "wp": np.ascontiguousarray(W_proj[i * D : (i + 1) * D, :]).astype(
                    ml_dtypes.bfloat16
                ),
            }
        )
    return in_maps


def kernel(x, W_attn, W_proj, _trace=False):
    from concourse.bass_utils import run_bass_kernel_spmd

    nc = _get_nc()
    in_maps = make_in_maps(
        np.asarray(x, dtype=np.float32),
        np.asarray(W_attn, dtype=np.float32),
        np.asarray(W_proj, dtype=np.float32),
    )
    res = run_bass_kernel_spmd(
        nc, in_maps, core_ids=list(range(NCORES)), trace=_trace
    )
    out = np.zeros((BT, C), dtype=np.float32)
    for r in res.results:
        out += r["outp"]
    out = out.reshape(B, T, C)
    if _trace:
        return out, res
    return out
